# revision 1
# baseline (speedup 1.0000x reference)
"""NT-Xent (SimCLR contrastive) loss on Trainium2, sharded across 8 NeuronCores.

Sharding: each core computes a [512, 4096] row-slice of the similarity matrix.
Host ships z^T (bf16, transposed layout only - no host arithmetic) shared to
all cores plus per-core own/partner column slices; per-core scalar partials
are summed on the host (the unshard step).

Device pipeline (per core, SPMD), interleaved per 1024-column block:
  - column norms: squares (DVE) + all-ones matmul (partition-reduce with free
    broadcast); rinv16 = exp(-0.5*ln(ssq) + ln 16)  [one ACT table set]
  - zn16 = z * rinv16 -> fp8e4 (x16 scaling keeps values clear of denormals)
  - block slice of the Gram matrix: fp8 DoubleRow matmuls, psum f32
  - exp((10/256)*sim256) row-sums fused into ScalarE's activation accumulator
  - diagonal mask: diag dot recomputed exactly (elementwise prod + ones-mm),
    moved to partition layout via K=1 outer-product matmuls, subtracted
    before the final ln
  - positives: elementwise prod + ones-mm, reduced along the free axis
"""

import numpy as np

B = 2048
D = 512
N2 = 2 * B              # 4096 total rows
NCORES = 8
RPC = N2 // NCORES      # 512 rows per core
KT = D // 128           # 4 contraction tiles
BLK = 1024              # column-block size
NBLK = N2 // BLK        # 4 blocks
TEMP = 0.1
SCALE = 1.0 / TEMP      # 10.0
FP8_SCALE = 16.0        # zn is stored as fp8(zn*16); sim256 = 256*sim
DVE_FP8_DIRECT = True   # False: bf16 + SWDGE cast-DMA (faster DVE, truncates)
# SWDGE dtype-cast truncates toward zero; pre-scale by ~half an average
# e4m3 ULP so the truncated grid is centered. Folded into the exp bias.
TRUNC_COMP = 1.0 if DVE_FP8_DIRECT else 1.045
LN_FP8_SCALE = float(np.log(FP8_SCALE * TRUNC_COMP))

_CACHE = {}


def _patch_act_tables(nc, mybir):
    """Make Ln and Exp resolve to the shared natural_log_exp_and_others set
    so the compiler emits one ACT table load instead of thrashing."""
    from concourse import hw_specs

    tables = hw_specs.get_activation_tables(nc.m.arch)
    keep = "natural_log_exp_and_others"
    if keep not in tables:
        return
    F = mybir.ActivationFunctionType
    if F.Exp not in tables[keep] or F.Ln not in tables[keep]:
        return
    for name, fns in tables.items():
        if name != keep:
            fns.discard(F.Exp)
            fns.discard(F.Ln)


def _build():
    from concourse import bass, bacc, tile, mybir

    nc = bacc.Bacc("TRN2", target_bir_lowering=False, debug=False,
                   num_devices=NCORES)
    bf16 = mybir.dt.bfloat16
    f32 = mybir.dt.float32
    f8 = mybir.dt.float8e4
    F = mybir.ActivationFunctionType
    A = mybir.AluOpType
    AX = mybir.AxisListType
    DR = mybir.MatmulPerfMode.DoubleRow
    PSUM = bass.MemorySpace.PSUM

    zt = nc.dram_tensor("zt", [D, N2], bf16, kind="ExternalInput").ap()
    zown = nc.dram_tensor("zown", [D, RPC], bf16, kind="ExternalInput").ap()
    zpr = nc.dram_tensor("zpr", [D, RPC], bf16, kind="ExternalInput").ap()
    out = nc.dram_tensor("out", [1, 1], f32, kind="ExternalOutput").ap()

    with tile.TileContext(nc) as tc:
        with (
            tc.tile_pool(name="sb", bufs=1) as sb,
            tc.tile_pool(name="wrk", bufs=2) as wrk,
            tc.tile_pool(name="psA", bufs=1, space=PSUM) as psA,
            tc.tile_pool(name="psA1", bufs=1, space=PSUM) as psA1,
            tc.tile_pool(name="psB", bufs=2, space=PSUM) as psB,
        ):
            ones = sb.tile([128, 128], bf16, tag="ones")
            nc.vector.memset(ones[:], 1.0)
            bias_ln16 = sb.tile([128, 1], f32, tag="bln16")
            nc.vector.memset(bias_ln16[:], LN_FP8_SCALE)
            bias_10 = sb.tile([128, 1], f32, tag="b10")
            nc.vector.memset(bias_10[:], SCALE)

            def load_slices(src, tag):
                ts = []
                for k in range(KT):
                    t = sb.tile([128, RPC], bf16, tag=f"{tag}{k}")
                    nc.sync.dma_start(out=t[:], in_=src[k * 128:(k + 1) * 128, :])
                    ts.append(t)
                return ts

            warm = psA1.tile([128, 512], f32, tag="pd")
            for _ in range(32):
                nc.tensor.matmul(warm[:, 0:128], ones[:], ones[:],
                                 start=True, stop=True)

            zok = load_slices(zown, "zo")
            zpk = load_slices(zpr, "zp")

            def to_fp8(zn_f8_tile, make_bf16_ap):
                """Write fp8 either directly from DVE or via bf16+cast DMA."""
                if DVE_FP8_DIRECT:
                    for k in range(KT):
                        make_bf16_ap(zn_f8_tile[:, k, :], k)
                else:
                    shape = list(zn_f8_tile.shape)
                    zn16 = wrk.tile(shape, bf16, tag="zn16")
                    for k in range(KT):
                        make_bf16_ap(zn16[:, k, :], k)
                    nc.gpsimd.dma_start(out=zn_f8_tile[:], in_=zn16[:])

            def norm_small(tks, tag, fp8_out):
                lns = wrk.tile([128, RPC], f32, tag="lns_s")
                rin = wrk.tile([128, RPC], bf16, tag="rin_s")
                ps = psA.tile([128, 512], f32, tag="ssq_s")
                for k in range(KT):
                    s = wrk.tile([128, RPC], bf16, tag="sq_s")
                    nc.vector.tensor_tensor(s[:], tks[k][:], tks[k][:], A.mult)
                    nc.tensor.matmul(ps[:], ones[:], s[:],
                                     start=(k == 0), stop=(k == KT - 1))
                nc.scalar.activation(lns[:], ps[:], F.Ln)
                bias = bias_ln16[:] if fp8_out else 0.0
                nc.scalar.activation(rin[:], lns[:], F.Exp, scale=-0.5, bias=bias)
                if fp8_out:
                    zn = sb.tile([128, KT, RPC], f8, tag=f"zn_{tag}")
                    to_fp8(zn, lambda ap, k: nc.vector.tensor_tensor(
                        ap, tks[k][:], rin[:], A.mult))
                else:
                    zn = sb.tile([128, KT, RPC], bf16, tag=f"zn_{tag}")
                    for k in range(KT):
                        nc.vector.tensor_tensor(zn[:, k, :], tks[k][:], rin[:],
                                                A.mult)
                return zn

            zno = norm_small(zok, "o", True)          # fp8(zn_own*16)
            znp = norm_small(zpk, "p", False)         # bf16, x1 scale

            # positives: prod = zno16 * znp = 16*zn*zn ; colsum-bcast
            pp = psA1.tile([128, 512], f32, tag="pd")
            for k in range(KT):
                pr = wrk.tile([128, RPC], bf16, tag="prod")
                nc.vector.tensor_tensor(pr[:], zno[:, k, :], znp[:, k, :],
                                        A.mult)
                nc.tensor.matmul(pp[:], ones[:], pr[:],
                                 start=(k == 0), stop=(k == KT - 1))
            pos_red = sb.tile([128, 1], f32, tag="posr")
            nc.vector.tensor_reduce(pos_red[:], pp[:], AX.X, A.add)

            # diag dots (one row): dg = sum_d (zn16)^2 = 256*|zn|^2
            dg = psA1.tile([1, 512], f32, tag="pd")
            for k in range(KT):
                pr = wrk.tile([128, RPC], bf16, tag="prod")
                nc.vector.tensor_tensor(pr[:], zno[:, k, :], zno[:, k, :],
                                        A.mult)
                nc.tensor.matmul(dg[:], ones[:, 0:1], pr[:],
                                 start=(k == 0), stop=(k == KT - 1))
            diag_row = sb.tile([1, RPC], bf16, tag="diagrow")
            nc.vector.tensor_scalar_add(diag_row[:], dg[:], -FP8_SCALE ** 2)

            # ---- per-block: normalize, then this block's matmuls + exp ----
            rowp = sb.tile([128, 4, NBLK], f32, tag="rowp")
            for b in range(NBLK):
                bsl = slice(b * BLK, (b + 1) * BLK)
                zb = sb.tile([128, KT, BLK], bf16, tag=f"zt{b}")
                for k in range(KT):
                    nc.sync.dma_start(out=zb[:, k, :],
                                      in_=zt[k * 128:(k + 1) * 128, bsl])
                sq = wrk.tile([128, KT, BLK], bf16, tag="sq")
                nc.vector.tensor_tensor(sq[:], zb[:], zb[:], A.mult)
                ps = psA.tile([128, BLK], f32, tag="ssq")
                for k in range(KT):
                    for j in range(BLK // 512):
                        nc.tensor.matmul(ps[:, j * 512:(j + 1) * 512],
                                         ones[:], sq[:, k, j * 512:(j + 1) * 512],
                                         start=(k == 0), stop=(k == KT - 1))
                lns = wrk.tile([128, BLK], f32, tag="lns")
                nc.scalar.activation(lns[:], ps[:], F.Ln)
                rin = wrk.tile([128, BLK], bf16, tag="rin")
                nc.scalar.activation(rin[:], lns[:], F.Exp, scale=-0.5,
                                     bias=bias_ln16[:])
                zn = sb.tile([128, KT, BLK], f8, tag=f"znt{b}")
                to_fp8(zn, lambda ap, k: nc.vector.tensor_tensor(
                    ap, zb[:, k, :], rin[:], A.mult))

                # this block's Gram columns + fused exp row-sums
                for m in range(4):
                    pm = psB.tile([128, BLK], f32, tag="mm")
                    for g in range(KT // 2):
                        lhsT = zno[:, 2 * g:2 * g + 2, m * 128:(m + 1) * 128]
                        for j in range(BLK // 512):
                            nc.tensor.matmul(
                                pm[:, j * 512:(j + 1) * 512],
                                lhsT,
                                zn[:, 2 * g:2 * g + 2, j * 512:(j + 1) * 512],
                                start=(g == 0), stop=(g == KT // 2 - 1),
                                perf_mode=DR)
                    scr = wrk.tile([128, BLK], f32, tag="scr")
                    nc.scalar.activation(scr[:], pm[:], F.Exp,
                                         scale=SCALE / (FP8_SCALE ** 2),
                                         accum_out=rowp[:, m, b:b + 1])

            # ---- finale: partial = sum_r ln(Z_r) - 10 * sum_r pos_r ----
            dt = psA1.tile([128, 512], f32, tag="pd")
            for m in range(4):
                nc.tensor.matmul(dt[:, m * 128:(m + 1) * 128],
                                 diag_row[0:1, m * 128:(m + 1) * 128],
                                 ones[0:1, :], start=True, stop=True)
            diag_part = sb.tile([128, 4], f32, tag="diagp")
            for m in range(4):
                nc.vector.tensor_copy(diag_part[:, m:m + 1],
                                      dt[:, m * 128:m * 128 + 1])
            dexp = sb.tile([128, 4], f32, tag="dexp")
            nc.scalar.activation(dexp[:], diag_part[:], F.Exp,
                                 scale=SCALE / (FP8_SCALE ** 2),
                                 bias=bias_10[:])
            zsum = sb.tile([128, 4], f32, tag="zsum")
            for m in range(4):
                nc.vector.tensor_reduce(zsum[:, m:m + 1], rowp[:, m, :],
                                        AX.X, A.add)
            zarg = sb.tile([128, 4], f32, tag="zarg")
            nc.vector.tensor_tensor(zarg[:], zsum[:], dexp[:], A.subtract)
            logz = sb.tile([128, 5], f32, tag="logz")
            nc.scalar.activation(logz[:, 0:4], zarg[:], F.Ln)
            nc.vector.tensor_scalar_mul(
                logz[:, 4:5], pos_red[:], -SCALE / FP8_SCALE / 128.0)
            red1 = sb.tile([128, 1], f32, tag="red1")
            nc.vector.tensor_reduce(red1[:], logz[:], AX.X, A.add)
            fin = sb.tile([1, 1], f32, tag="fin")
            nc.gpsimd.tensor_reduce(fin[:], red1[:], AX.C, A.add)
            nc.sync.dma_start(out=out, in_=fin[:])

    _patch_act_tables(nc, mybir)
    nc.compile()
    return nc


def _get_nc():
    if "nc" not in _CACHE:
        _CACHE["nc"] = _build()
    return _CACHE["nc"]


def _in_maps(z_i, z_j):
    import ml_dtypes

    z = np.concatenate(
        [np.asarray(z_i, np.float32), np.asarray(z_j, np.float32)], axis=0)
    zt = np.ascontiguousarray(z.T).astype(ml_dtypes.bfloat16)
    maps = []
    for c in range(NCORES):
        o = c * RPC
        po = (o + B) % N2
        maps.append({
            "zt": zt,
            "zown": np.ascontiguousarray(zt[:, o:o + RPC]),
            "zpr": np.ascontiguousarray(zt[:, po:po + RPC]),
        })
    return maps


def _run(z_i, z_j, trace=False):
    from concourse.bass_utils import run_bass_kernel_spmd

    nc = _get_nc()
    return run_bass_kernel_spmd(nc, _in_maps(z_i, z_j), list(range(NCORES)),
                                trace=trace)


def kernel(z_i, z_j):
    res = _run(z_i, z_j, trace=False)
    total = sum(float(r["out"][0, 0]) for r in res.results)
    return np.float32(total / N2)

